# revision 20
# baseline (speedup 1.0000x reference)
"""Trainium2 Bass kernel for DiagonalKernelAverageV2.

Math: for each (b, ch) image X [512, 512] and each of 4 corners, the output
at index i is the mean over the L-shaped shell of the i-th nested corner
square:  shell[i] = d[i] - d[i-1],  d[i] = sum of the (i+1)x(i+1) corner
window,  counts[i] = 2i+1.

Only two shell families are computed directly (top-left and top-right); the
bottom corners follow from row/col totals:
    shell_tl[i] = sum_{c<=i} X[i,c] + sum_{r<i}  X[r,i]
    shell_tr[i] = sum_{c>=511-i} X[i,c] + sum_{r<i} X[r,511-i]
    shell_br[i] = S[511-i] + ST[511-i] - shell_tl[511-i]
    shell_bl[i] = S[511-i] + ST[i]     - shell_tr[511-i]
(S = row sums, ST = col sums.)

Per-core layout: batch-sharded (4 batches x 8 channels per core).  Each image
is 4 row-tiles [128, 512].  Per image:
  - VectorE: one segmented reduce -> 16 block row-sums B[t][j]; 8 fused
    tensor_tensor_reduce ops with a strict-upper mask on the diagonal /
    (reversed) antidiagonal 128x128 blocks -> masked products (P1, P2rev) +
    their row sums.
  - TensorE: per-tile matmuls with constant-column weights accumulate column
    prefix sums / totals; ones-matmuls over P1/P2rev give the within-block
    column partial sums; 4 transposes move column-indexed rows onto
    partitions.
  - ScalarE: PSUM->SBUF staging copies (incl. free-dim-reversed copies).
Bottom-corner outputs are written in source order and flipped on the host.
"""

import numpy as np

SIZE = 512
NT = 4  # row tiles per image
NCH = 8  # channels per batch
NB_CORE = 4  # batches per core
N_CORES = 8
NQ = 10  # T_in rows
DBG_STAGE = 2  # debug aid: 1 = per-image pipeline only, 2 = full kernel


def build_nc():
    import concourse.bass as bass
    import concourse.bacc as bacc
    import concourse.mybir as mybir
    from concourse.tile import TileContext

    f32 = mybir.dt.float32
    f16 = mybir.dt.float16
    nc = bacc.Bacc()

    x = nc.dram_tensor("x", [NB_CORE, NCH, SIZE, SIZE], f16, kind="ExternalInput")
    msu_d = nc.dram_tensor("msu", [128, 4 * 128], f16, kind="ExternalInput")
    vw_d = nc.dram_tensor("vw", [128, 36], f16, kind="ExternalInput")
    eye_d = nc.dram_tensor("eye", [128, 128], f32, kind="ExternalInput")
    ones_d = nc.dram_tensor("onesv", [128, 1], f32, kind="ExternalInput")
    wg_d = nc.dram_tensor("wg", [128, NCH, NT], f32, kind="ExternalInput")
    wrevg_d = nc.dram_tensor("wrevg", [128, NCH, NT], f32, kind="ExternalInput")
    out = nc.dram_tensor("out", [NB_CORE, SIZE, 4 * NCH], f32, kind="ExternalOutput")
    if DBG_STAGE < 2:
        dbg_tq = nc.dram_tensor(
            "dbg_tq", [NB_CORE, 128, NCH * NT * NQ], f32, kind="ExternalOutput"
        )
        dbg_b = nc.dram_tensor(
            "dbg_b", [NB_CORE, 128, NCH * NT * NT], f32, kind="ExternalOutput"
        )

    ADD = mybir.AluOpType.add
    MULT = mybir.AluOpType.mult
    SUB = mybir.AluOpType.subtract
    AX = mybir.AxisListType.X

    with TileContext(nc) as tc, nc.allow_low_precision(reason="fp16 inputs"):
        with (
            tc.tile_pool(name="consts", bufs=1) as consts,
            tc.tile_pool(name="xs", bufs=3) as xpool,
            tc.tile_pool(name="tree", bufs=2) as trpool,
            tc.tile_pool(name="tin", bufs=2) as tinpool,
            tc.tile_pool(name="perb", bufs=2) as bpool,
            tc.tile_pool(name="small", bufs=2) as spool,
            tc.tile_pool(name="psq", bufs=2, space="PSUM") as psq,
            tc.tile_pool(name="pst", bufs=2, space="PSUM") as pst,
        ):
            msu = consts.tile([128, 4 * 128], f16)
            nc.sync.dma_start(out=msu, in_=msu_d[:])
            vw = consts.tile([128, 36], f16)
            nc.sync.dma_start(out=vw, in_=vw_d[:])
            eye = consts.tile([128, 128], f32)
            nc.sync.dma_start(out=eye, in_=eye_d[:])
            onev = consts.tile([128, 1], f32)
            nc.sync.dma_start(out=onev, in_=ones_d[:])
            wg = consts.tile([128, NCH, NT], f32)
            nc.sync.dma_start(out=wg, in_=wg_d[:])
            wrevg = consts.tile([128, NCH, NT], f32)
            nc.sync.dma_start(out=wrevg, in_=wrevg_d[:])
            msu4 = msu.rearrange("p (t c) -> p t c", c=128)

            from concourse.bass import _add_dep_helper

            prev_pe_last = None
            for b in range(NB_CORE):
                # B24[p, g, k]: k=4t+j -> block sum B[t][j]; k=16+t -> RSsu[t];
                # k=20+t -> RS2su[t]
                B24 = bpool.tile([128, NCH, 24], f32, tag="b24")
                TQ = bpool.tile([128, NCH, NT, NQ], f32, tag="tq")

                for g in range(NCH):
                    # XP: 24 blocks of [128, 128]: 0-15 = X (t-major), 16-19 =
                    # P1 (strict-upper-masked diag blocks), 20-23 = P2rev
                    # (strict-upper-masked reversed antidiag blocks).
                    XP = xpool.tile([128, 24, 128], f16)
                    nc.sync.dma_start(
                        out=XP[:, 0:16, :].rearrange("p (t j) c -> p t (j c)", t=NT),
                        in_=x[b, g].rearrange("(t p) c -> p t c", p=128),
                    )
                    XPf = XP.rearrange("p a b -> p (a b)")

                    def blk_ap(base, tstep, cstep=1, coff=0):
                        return bass.AP(
                            tensor=XP.tensor,
                            offset=XP[:, 0, 0:1].offset + base * 128 + coff,
                            ap=[XP[:, 0, 0:1].ap[0]] + [[tstep * 128, NT], [cstep, 128]],
                        )

                    # P1 products on DVE (fp16 2x); P2rev products on GpSimd
                    nc.vector.tensor_tensor(
                        XP[:, 16:20, :], blk_ap(0, 5), msu4, op=MULT
                    )
                    nc.gpsimd.tensor_tensor(
                        XP[:, 20:24, :], blk_ap(3, 3, cstep=-1, coff=127), msu4,
                        op=MULT,
                    )
                    # 24 block row sums via fp16 2x pairwise-add tree
                    # (tensor_tensor gets 2x on packed fp16; tensor_reduce does
                    # not); final 8-wide reduce on GpSimd -> B24[:, g].
                    T1 = trpool.tile([128, 24, 64], f16, tag="t1")
                    T2 = trpool.tile([128, 24, 32], f16, tag="t2")
                    T3 = trpool.tile([128, 24, 16], f16, tag="t3")
                    T4 = trpool.tile([128, 24, 8], f16, tag="t4")
                    nc.vector.tensor_tensor(T1, XP[:, :, 0:64], XP[:, :, 64:128], op=ADD)
                    nc.vector.tensor_tensor(T2, T1[:, :, 0:32], T1[:, :, 32:64], op=ADD)
                    nc.vector.tensor_tensor(T3, T2[:, :, 0:16], T2[:, :, 16:32], op=ADD)
                    nc.vector.tensor_tensor(T4, T3[:, :, 0:8], T3[:, :, 8:16], op=ADD)
                    # last 3 halvings on GpSimd (DVE is the bottleneck engine)
                    T5 = trpool.tile([128, 24, 4], f16, tag="t5")
                    T6 = trpool.tile([128, 24, 2], f16, tag="t6")
                    nc.gpsimd.tensor_tensor(T5, T4[:, :, 0:4], T4[:, :, 4:8], op=ADD)
                    nc.gpsimd.tensor_tensor(T6, T5[:, :, 0:2], T5[:, :, 2:4], op=ADD)
                    nc.gpsimd.tensor_tensor(
                        B24[:, g], T6[:, :, 0], T6[:, :, 1], op=ADD
                    )
                    # column-side quantities on PE: one accumulation group
                    # rows 0-2: CPfx[1..3], 3: ST, 4: colsum(P1), 5: colsum(P2rev)
                    psumQ = psq.tile([6, SIZE], f32)
                    for t in range(NT):
                        mm = nc.tensor.matmul(
                            psumQ[0:6, :],
                            lhsT=vw[:, 6 * t : 6 * t + 6],
                            rhs=XPf[:, 512 * t : 512 * (t + 1)],
                            start=(t == 0),
                            stop=False,
                        )
                        # keep PE program order: no transpose-mode matmul from a
                        # previous image may interleave into this accum group
                        if t == 0 and prev_pe_last is not None:
                            _add_dep_helper(
                                mm.ins, prev_pe_last.ins, sync=False,
                                reason="PE group ordering",
                            )
                    nc.tensor.matmul(
                        psumQ[0:6, :], lhsT=vw[:, 24:30], rhs=XPf[:, 2048:2560],
                        start=False, stop=False,
                    )
                    nc.tensor.matmul(
                        psumQ[0:6, :], lhsT=vw[:, 30:36], rhs=XPf[:, 2560:3072],
                        start=False, stop=True,
                    )
                    # stage to SBUF: direct rows and free-reversed rows, both
                    # in partition-base-0 tiles (base-32 transpose inputs
                    # crash the PE after repeated use)
                    Tin = tinpool.tile([6, SIZE], f32)
                    TinB = tinpool.tile([4, SIZE], f32)
                    nc.scalar.copy(Tin[0:6, :], psumQ[0:6, :])
                    nc.scalar.copy(TinB[0:4, :], psumQ[0:4, ::-1])
                    # transpose T_in blocks -> quantities on partitions
                    psumT = pst.tile([128, NT * NQ], f32)
                    for t in range(NT):
                        nc.tensor.transpose(
                            psumT[:, NQ * t : NQ * t + 6],
                            in_=Tin[0:6, 128 * t : 128 * (t + 1)],
                            identity=eye[0:6, 0:6],
                        )
                        prev_pe_last = nc.tensor.transpose(
                            psumT[:, NQ * t + 6 : NQ * t + 10],
                            in_=TinB[0:4, 128 * t : 128 * (t + 1)],
                            identity=eye[0:4, 0:4],
                        )
                    nc.scalar.copy(
                        TQ[:, g].rearrange("p t q -> p (t q)"), psumT[:, :]
                    )

                if DBG_STAGE == 1:
                    nc.sync.dma_start(
                        out=dbg_tq[b], in_=TQ.rearrange("p a b c -> p (a b c)")
                    )
                    nc.sync.dma_start(
                        out=dbg_b[b], in_=B_G.rearrange("p a b c -> p (a b c)")
                    )
                    continue
                # ---- per-batch assembly (all [128, (g), (t)] strided ops) ----
                def bg_ap(base, tstep):
                    return bass.AP(
                        tensor=B24.tensor,
                        offset=B24[:, 0, 0:1].offset + base,
                        ap=[B24[:, 0, 0:1].ap[0]] + [[24, NCH], [tstep, NT]],
                    )

                def tq_ap(base, tstep, nt=NT):
                    return bass.AP(
                        tensor=TQ.tensor,
                        offset=TQ[:, 0, 0, 0:1].offset + base,
                        ap=[TQ[:, 0, 0, 0:1].ap[0]] + [[NT * NQ, NCH], [tstep, nt]],
                    )

                PI = bpool.tile([128, NCH, 5, NT], f32, tag="pi")

                def pi_ap(base, tstep, nt=NT):
                    return bass.AP(
                        tensor=PI.tensor,
                        offset=PI[:, 0, 0, 0:1].offset + base,
                        ap=[PI[:, 0, 0, 0:1].ap[0]] + [[20, NCH], [tstep, nt]],
                    )

                nc.vector.memset(PI[:, :, 0, :], 0.0)
                nc.vector.tensor_copy(PI[:, :, 1, :], bg_ap(0, 4))
                for m in range(2, 5):
                    nc.vector.tensor_tensor(
                        PI[:, :, m, :], PI[:, :, m - 1, :], bg_ap(m - 1, 4),
                        op=ADD,
                    )

                sh_tl = spool.tile([128, NCH, NT], f32, tag="shtl")
                sh_tr = spool.tile([128, NCH, NT], f32, tag="shtr")
                # shell_tl = B[t][t] - RSsu + PI[m=t] + CPfx[m=t] + CS1
                nc.vector.tensor_tensor(sh_tl, bg_ap(0, 5), bg_ap(16, 1), op=SUB)
                nc.vector.tensor_tensor(sh_tl, sh_tl, pi_ap(0, 5), op=ADD)
                nc.vector.tensor_tensor(
                    sh_tl[:, :, 1:4], sh_tl[:, :, 1:4], tq_ap(NQ, NQ + 1, 3), op=ADD
                )
                nc.vector.tensor_tensor(sh_tl, sh_tl, tq_ap(4, NQ), op=ADD)
                # shell_tr = B[t][3-t] - RS2su + S - PI[m=4-t] + CPfxRev[m=t] + CS2
                nc.vector.tensor_tensor(sh_tr, bg_ap(3, 3), bg_ap(20, 1), op=SUB)
                nc.vector.tensor_tensor(sh_tr, sh_tr, pi_ap(16, 1), op=ADD)
                nc.vector.tensor_tensor(sh_tr, sh_tr, pi_ap(16, -3), op=SUB)
                nc.vector.tensor_tensor(
                    sh_tr[:, :, 1:4], sh_tr[:, :, 1:4], tq_ap(NQ + 6, NQ + 1, 3),
                    op=ADD,
                )
                nc.vector.tensor_tensor(sh_tr, sh_tr, tq_ap(5, NQ), op=ADD)

                if DBG_STAGE == 1.5:
                    nc.vector.tensor_copy(
                        TQ[:, 0, 0, 0:4], sh_tl[:, 0, :]
                    )
                    nc.vector.tensor_copy(
                        TQ[:, 0, 1, 0:4], sh_tr[:, 0, :]
                    )
                    nc.sync.dma_start(
                        out=dbg_tq[b], in_=TQ.rearrange("p a b c -> p (a b c)")
                    )
                    nc.sync.dma_start(
                        out=dbg_b[b], in_=B_G.rearrange("p a b c -> p (a b c)")
                    )
                    continue
                # br (src order): u = ST - shell_tl + S ; bl: v = STrev - shell_tr + S
                u = spool.tile([128, NCH, NT], f32, tag="u")
                v = spool.tile([128, NCH, NT], f32, tag="v")
                nc.vector.tensor_tensor(u, tq_ap(3, NQ), sh_tl, op=SUB)
                nc.vector.tensor_tensor(u, u, pi_ap(16, 1), op=ADD)
                nc.vector.tensor_tensor(v, tq_ap(9, NQ), sh_tr, op=SUB)
                nc.vector.tensor_tensor(v, v, pi_ap(16, 1), op=ADD)
                # outputs as [128, t, g] tiles, weighted; one DMA per corner
                outv = out[b].rearrange("(t p) c -> p t c", p=128)
                for ci, (src, wt) in enumerate(
                    [(sh_tl, wg), (sh_tr, wg), (v, wrevg), (u, wrevg)]
                ):
                    o_c = spool.tile([128, NT, NCH], f32, tag=f"oc{ci}")
                    nc.vector.tensor_tensor(
                        o_c,
                        src.rearrange("p g t -> p t g"),
                        wt.rearrange("p g t -> p t g"),
                        op=MULT,
                    )
                    nc.sync.dma_start(
                        out=outv[:, :, ci * NCH : (ci + 1) * NCH], in_=o_c
                    )
    nc.compile()
    return nc


def make_consts():
    r = np.arange(128)
    msu = np.tile((r[None, :] > r[:, None]).astype(np.float16), (1, 4))  # [c > r]
    vw = np.zeros((128, 36), np.float16)
    for t in range(NT):
        for m in range(3):
            vw[:, 6 * t + m] = 1.0 if t < m + 1 else 0.0  # CPfx[m+1]
        vw[:, 6 * t + 3] = 1.0  # ST
    vw[:, 24 + 4] = 1.0  # colsum(P1) -> row 4
    vw[:, 30 + 5] = 1.0  # colsum(P2rev) -> row 5
    eye = np.eye(128, dtype=np.float32)
    onesv = np.ones((128, 1), np.float32)
    i_pt = (r[:, None] + 128 * np.arange(NT)[None, :]).astype(np.float64)
    w_pt = (1.0 / (2 * i_pt + 1)).astype(np.float32)  # [128, NT]
    wrev_pt = (1.0 / (1023.0 - 2 * i_pt)).astype(np.float32)
    wg = np.tile(w_pt[:, None, :], (1, NCH, 1)).astype(np.float32)
    wrevg = np.tile(wrev_pt[:, None, :], (1, NCH, 1)).astype(np.float32)
    return dict(msu=msu, vw=vw, eye=eye, onesv=onesv, wg=wg, wrevg=wrevg)


_NC = None


def _get_nc():
    global _NC
    if _NC is None:
        _NC = build_nc()
    return _NC


def kernel(x: np.ndarray) -> np.ndarray:
    from concourse.bass_utils import run_bass_kernel_spmd

    x = np.asarray(x, dtype=np.float32).astype(np.float16)
    B = x.shape[0]
    consts = make_consts()
    per_core = B // N_CORES
    assert per_core == NB_CORE
    in_maps = [
        {"x": x[c * per_core : (c + 1) * per_core], **consts}
        for c in range(N_CORES)
    ]
    nc = _get_nc()
    res = run_bass_kernel_spmd(nc, in_maps, core_ids=list(range(N_CORES)))
    outs = []
    for r in res.results:
        o = r["out"].copy()  # [NB_CORE, 512, 4*NCH]
        o[:, :, 2 * NCH :] = o[:, ::-1, 2 * NCH :]
        outs.append(o)
    return np.concatenate(outs, axis=0)



# revision 23
# speedup vs baseline: 1.0226x; 1.0226x over previous
"""Trainium2 Bass kernel for DiagonalKernelAverageV2.

Math: for each (b, ch) image X [512, 512] and each of 4 corners, the output
at index i is the mean over the L-shaped shell of the i-th nested corner
square:  shell[i] = d[i] - d[i-1],  d[i] = sum of the (i+1)x(i+1) corner
window,  counts[i] = 2i+1.

Only two shell families are computed directly (top-left and top-right); the
bottom corners follow from row/col totals:
    shell_tl[i] = sum_{c<=i} X[i,c] + sum_{r<i}  X[r,i]
    shell_tr[i] = sum_{c>=511-i} X[i,c] + sum_{r<i} X[r,511-i]
    shell_br[i] = S[511-i] + ST[511-i] - shell_tl[511-i]
    shell_bl[i] = S[511-i] + ST[i]     - shell_tr[511-i]
(S = row sums, ST = col sums.)

Inputs are fed to the device as fp16 (quantization rel-err ~2e-4, well under
the 2e-2 gate); this halves HBM traffic and runs every PE matmul at the
1-cycle/row rate.

Per-core layout: batch-sharded (4 batches x 8 channels per core).  Each image
is 4 row-tiles [128, 512], held as 24 x [128, 128] blocks per image in XP:
blocks 0-15 = X (t-major), 16-19 = P1 (strict-upper-masked diagonal blocks),
20-23 = P2rev (strict-upper-masked reversed antidiagonal blocks).  Work split
by engine, per pair of images:
  - GpSimd: masked products (writes XP blocks 16-23); tail of the block-sum
    tree (widths 16 -> 1) producing B24 = 16 block sums + RSsu + RS2su per
    image; all per-batch assembly and output weighting.
  - VectorE: first two levels of the pairwise-add block-sum tree (fp16 2x).
  - TensorE: per-tile matmuls with constant-column weights accumulate column
    prefix sums / totals; ones-matmuls over P1/P2rev give the within-block
    column partial sums; transposes (incl. reversed-stride views) move
    column-indexed rows onto partitions.
  - ScalarE: PSUM->SBUF staging copies.
Bottom-corner outputs are written in source order and flipped on the host.
"""

import numpy as np

SIZE = 512
NT = 4  # row tiles per image
NCH = 8  # channels per batch
NB_CORE = 4  # batches per core
N_CORES = 8
NQ = 10  # transposed quantity cols per tile: 6 fwd + 4 reversed
NPAIR = NCH // 2


def build_nc():
    import concourse.bass as bass
    import concourse.bacc as bacc
    import concourse.mybir as mybir
    from concourse.tile import TileContext

    f32 = mybir.dt.float32
    f16 = mybir.dt.float16
    nc = bacc.Bacc()

    x = nc.dram_tensor("x", [NB_CORE, NCH, SIZE, SIZE], f16, kind="ExternalInput")
    msu_d = nc.dram_tensor("msu", [128, 4 * 128], f16, kind="ExternalInput")
    vw_d = nc.dram_tensor("vw", [128, 36], f16, kind="ExternalInput")
    eye_d = nc.dram_tensor("eye", [128, 128], f32, kind="ExternalInput")
    wg_d = nc.dram_tensor("wg", [128, NCH, NT], f32, kind="ExternalInput")
    wrevg_d = nc.dram_tensor("wrevg", [128, NCH, NT], f32, kind="ExternalInput")
    out = nc.dram_tensor("out", [NB_CORE, SIZE, 4 * NCH], f32, kind="ExternalOutput")

    ADD = mybir.AluOpType.add
    MULT = mybir.AluOpType.mult
    SUB = mybir.AluOpType.subtract

    with TileContext(nc) as tc, nc.allow_low_precision(reason="fp16 pipeline"):
        with (
            tc.tile_pool(name="consts", bufs=1) as consts,
            tc.tile_pool(name="xs", bufs=3) as xpool,
            tc.tile_pool(name="tree", bufs=2) as trpool,
            tc.tile_pool(name="tin", bufs=2) as tinpool,
            tc.tile_pool(name="perb", bufs=2) as bpool,
            tc.tile_pool(name="small", bufs=2) as spool,
            tc.tile_pool(name="psq", bufs=2, space="PSUM") as psq,
            tc.tile_pool(name="pst", bufs=2, space="PSUM") as pst,
        ):
            msu = consts.tile([128, 4 * 128], f16)
            nc.sync.dma_start(out=msu, in_=msu_d[:])
            vw = consts.tile([128, 36], f16)
            nc.sync.dma_start(out=vw, in_=vw_d[:])
            eye = consts.tile([128, 128], f32)
            nc.sync.dma_start(out=eye, in_=eye_d[:])
            wg = consts.tile([128, NCH, NT], f32)
            nc.sync.dma_start(out=wg, in_=wg_d[:])
            wrevg = consts.tile([128, NCH, NT], f32)
            nc.sync.dma_start(out=wrevg, in_=wrevg_d[:])
            msu4 = msu.rearrange("p (t c) -> p t c", c=128)

            from concourse.bass import _add_dep_helper

            prev_pe_last = None
            for b in range(NB_CORE):
                # B24[p, g, k]: k=4t+j -> block sum B[t][j]; k=16+t -> RSsu[t];
                # k=20+t -> RS2su[t]
                B24 = bpool.tile([128, NCH, 24], f32, tag="b24")
                TQ = bpool.tile([128, NCH, NT, NQ], f32, tag="tq")

                for gp in range(NPAIR):
                    XP = xpool.tile([128, 2, 24, 128], f16)
                    for i in (0, 1):
                        nc.sync.dma_start(
                            out=XP[:, i, 0:16, :].rearrange(
                                "p (t j) c -> p t (j c)", t=NT
                            ),
                            in_=x[b, 2 * gp + i].rearrange("(t p) c -> p t c", p=128),
                        )

                    # masked products on GpSimd -> XP blocks 16-23
                    for i in (0, 1):
                        off_i = i * 24 * 128

                        def blk_ap(base, tstep, cstep=1, coff=0):
                            return bass.AP(
                                tensor=XP.tensor,
                                offset=XP[:, 0, 0, 0:1].offset
                                + off_i + base * 128 + coff,
                                ap=[XP[:, 0, 0, 0:1].ap[0]]
                                + [[tstep * 128, NT], [cstep, 128]],
                            )

                        nc.gpsimd.tensor_tensor(
                            XP[:, i, 16:20, :], blk_ap(0, 5), msu4, op=MULT
                        )
                        nc.gpsimd.tensor_tensor(
                            XP[:, i, 20:24, :],
                            blk_ap(3, 3, cstep=-1, coff=127),
                            msu4,
                            op=MULT,
                        )

                    # block row sums: fp16 2x pairwise-add tree; first two
                    # levels on VectorE, tail on GpSimd
                    T1 = trpool.tile([128, 2, 24, 64], f16, tag="t1")
                    T2 = trpool.tile([128, 2, 24, 32], f16, tag="t2")
                    T3 = trpool.tile([128, 2, 24, 16], f16, tag="t3")
                    T4 = trpool.tile([128, 2, 24, 8], f16, tag="t4")
                    T5 = trpool.tile([128, 2, 24, 4], f16, tag="t5")
                    T6 = trpool.tile([128, 2, 24, 2], f16, tag="t6")
                    nc.vector.tensor_tensor(
                        T1, XP[:, :, :, 0:64], XP[:, :, :, 64:128], op=ADD
                    )
                    nc.vector.tensor_tensor(
                        T2, T1[:, :, :, 0:32], T1[:, :, :, 32:64], op=ADD
                    )
                    nc.gpsimd.tensor_tensor(
                        T3, T2[:, :, :, 0:16], T2[:, :, :, 16:32], op=ADD
                    )
                    nc.gpsimd.tensor_tensor(
                        T4, T3[:, :, :, 0:8], T3[:, :, :, 8:16], op=ADD
                    )
                    nc.gpsimd.tensor_tensor(
                        T5, T4[:, :, :, 0:4], T4[:, :, :, 4:8], op=ADD
                    )
                    nc.gpsimd.tensor_tensor(
                        T6, T5[:, :, :, 0:2], T5[:, :, :, 2:4], op=ADD
                    )
                    nc.gpsimd.tensor_tensor(
                        B24[:, 2 * gp : 2 * gp + 2],
                        T6[:, :, :, 0],
                        T6[:, :, :, 1],
                        op=ADD,
                    )

                    # column-side quantities on PE, per image: one accum group
                    # rows 0-2: CPfx[1..3], 3: ST, 4: colsum(P1), 5: colsum(P2rev)
                    for i in (0, 1):
                        g = 2 * gp + i
                        XPi = XP[:, i].rearrange("p a b -> p (a b)")
                        psumQ = psq.tile([6, SIZE], f32)
                        for t in range(NT):
                            mm = nc.tensor.matmul(
                                psumQ[0:6, :],
                                lhsT=vw[:, 6 * t : 6 * t + 6],
                                rhs=XPi[:, 512 * t : 512 * (t + 1)],
                                start=(t == 0),
                                stop=False,
                            )
                            # keep PE program order: no transpose-mode matmul
                            # from a previous image may interleave into this
                            # accum group
                            if t == 0 and prev_pe_last is not None:
                                _add_dep_helper(
                                    mm.ins, prev_pe_last.ins, sync=False,
                                    reason="PE group ordering",
                                )
                        nc.tensor.matmul(
                            psumQ[0:6, :], lhsT=vw[:, 24:30],
                            rhs=XPi[:, 2048:2560], start=False, stop=False,
                        )
                        nc.tensor.matmul(
                            psumQ[0:6, :], lhsT=vw[:, 30:36],
                            rhs=XPi[:, 2560:3072], start=False, stop=True,
                        )
                        # stage to SBUF (partition-base-0 tile), then transpose
                        # quantity blocks onto partitions; reversed quantities
                        # use a reversed-stride view of Tin (no extra copy)
                        Tin = tinpool.tile([6, SIZE], f32)
                        TinB = tinpool.tile([4, SIZE], f32)
                        nc.scalar.copy(Tin[0:6, :], psumQ[0:6, :])
                        nc.scalar.copy(TinB[0:4, :], psumQ[0:4, ::-1])
                        psumT = pst.tile([128, NT * NQ], f32)
                        for t in range(NT):
                            nc.tensor.transpose(
                                psumT[:, NQ * t : NQ * t + 6],
                                in_=Tin[0:6, 128 * t : 128 * (t + 1)],
                                identity=eye[0:6, 0:6],
                            )
                            prev_pe_last = nc.tensor.transpose(
                                psumT[:, NQ * t + 6 : NQ * t + 10],
                                in_=TinB[0:4, 128 * t : 128 * (t + 1)],
                                identity=eye[0:4, 0:4],
                            )
                        nc.scalar.copy(
                            TQ[:, g].rearrange("p t q -> p (t q)"), psumT[:, :]
                        )

                # ---- per-batch assembly on GpSimd ([128, (g), (t)] ops) ----
                def bg_ap(base, tstep):
                    return bass.AP(
                        tensor=B24.tensor,
                        offset=B24[:, 0, 0:1].offset + base,
                        ap=[B24[:, 0, 0:1].ap[0]] + [[24, NCH], [tstep, NT]],
                    )

                def tq_ap(base, tstep, nt=NT):
                    return bass.AP(
                        tensor=TQ.tensor,
                        offset=TQ[:, 0, 0, 0:1].offset + base,
                        ap=[TQ[:, 0, 0, 0:1].ap[0]] + [[NT * NQ, NCH], [tstep, nt]],
                    )

                PI = bpool.tile([128, NCH, 5, NT], f32, tag="pi")

                def pi_ap(base, tstep, nt=NT):
                    return bass.AP(
                        tensor=PI.tensor,
                        offset=PI[:, 0, 0, 0:1].offset + base,
                        ap=[PI[:, 0, 0, 0:1].ap[0]] + [[20, NCH], [tstep, nt]],
                    )

                nc.gpsimd.memset(PI[:, :, 0, :], 0.0)
                nc.gpsimd.tensor_copy(PI[:, :, 1, :], bg_ap(0, 4))
                for m in range(2, 5):
                    nc.gpsimd.tensor_tensor(
                        PI[:, :, m, :], PI[:, :, m - 1, :], bg_ap(m - 1, 4),
                        op=ADD,
                    )

                sh_tl = spool.tile([128, NCH, NT], f32, tag="shtl")
                sh_tr = spool.tile([128, NCH, NT], f32, tag="shtr")
                # shell_tl = B[t][t] - RSsu + PI[m=t] + CPfx[m=t] + CS1
                nc.gpsimd.tensor_tensor(sh_tl, bg_ap(0, 5), bg_ap(16, 1), op=SUB)
                nc.gpsimd.tensor_tensor(sh_tl, sh_tl, pi_ap(0, 5), op=ADD)
                nc.gpsimd.tensor_tensor(
                    sh_tl[:, :, 1:4], sh_tl[:, :, 1:4], tq_ap(NQ, NQ + 1, 3), op=ADD
                )
                nc.gpsimd.tensor_tensor(sh_tl, sh_tl, tq_ap(4, NQ), op=ADD)
                # shell_tr = B[t][3-t] - RS2su + S - PI[m=4-t] + CPfxRev[m=t] + CS2
                nc.gpsimd.tensor_tensor(sh_tr, bg_ap(3, 3), bg_ap(20, 1), op=SUB)
                nc.gpsimd.tensor_tensor(sh_tr, sh_tr, pi_ap(16, 1), op=ADD)
                nc.gpsimd.tensor_tensor(sh_tr, sh_tr, pi_ap(16, -3), op=SUB)
                nc.gpsimd.tensor_tensor(
                    sh_tr[:, :, 1:4], sh_tr[:, :, 1:4], tq_ap(NQ + 6, NQ + 1, 3),
                    op=ADD,
                )
                nc.gpsimd.tensor_tensor(sh_tr, sh_tr, tq_ap(5, NQ), op=ADD)

                # br (src order): u = ST - shell_tl + S ; bl: v = STrev - shell_tr + S
                u = spool.tile([128, NCH, NT], f32, tag="u")
                v = spool.tile([128, NCH, NT], f32, tag="v")
                nc.gpsimd.tensor_tensor(u, tq_ap(3, NQ), sh_tl, op=SUB)
                nc.gpsimd.tensor_tensor(u, u, pi_ap(16, 1), op=ADD)
                nc.gpsimd.tensor_tensor(v, tq_ap(9, NQ), sh_tr, op=SUB)
                nc.gpsimd.tensor_tensor(v, v, pi_ap(16, 1), op=ADD)
                # outputs as one [128, t, ci, g] tile, weighted; one DMA per batch
                o_all = spool.tile([128, NT, 4, NCH], f32, tag="oall")
                for ci, (src, wt) in enumerate(
                    [(sh_tl, wg), (sh_tr, wg), (v, wrevg), (u, wrevg)]
                ):
                    nc.gpsimd.tensor_tensor(
                        o_all[:, :, ci, :],
                        src.rearrange("p g t -> p t g"),
                        wt.rearrange("p g t -> p t g"),
                        op=MULT,
                    )
                nc.sync.dma_start(
                    out=out[b].rearrange("(t p) c -> p t c", p=128),
                    in_=o_all.rearrange("p t c g -> p t (c g)"),
                )
    nc.compile()
    return nc


def make_consts():
    r = np.arange(128)
    msu = np.tile((r[None, :] > r[:, None]).astype(np.float16), (1, 4))  # [c > r]
    vw = np.zeros((128, 36), np.float16)
    for t in range(NT):
        for m in range(3):
            vw[:, 6 * t + m] = 1.0 if t < m + 1 else 0.0  # CPfx[m+1]
        vw[:, 6 * t + 3] = 1.0  # ST
    vw[:, 24 + 4] = 1.0  # colsum(P1) -> row 4
    vw[:, 30 + 5] = 1.0  # colsum(P2rev) -> row 5
    eye = np.eye(128, dtype=np.float32)
    i_pt = (r[:, None] + 128 * np.arange(NT)[None, :]).astype(np.float64)
    w_pt = (1.0 / (2 * i_pt + 1)).astype(np.float32)  # [128, NT]
    wrev_pt = (1.0 / (1023.0 - 2 * i_pt)).astype(np.float32)
    wg = np.tile(w_pt[:, None, :], (1, NCH, 1)).astype(np.float32)
    wrevg = np.tile(wrev_pt[:, None, :], (1, NCH, 1)).astype(np.float32)
    return dict(msu=msu, vw=vw, eye=eye, wg=wg, wrevg=wrevg)


_NC = None


def _get_nc():
    global _NC
    if _NC is None:
        _NC = build_nc()
    return _NC


def kernel(x: np.ndarray) -> np.ndarray:
    from concourse.bass_utils import run_bass_kernel_spmd

    x = np.asarray(x, dtype=np.float32).astype(np.float16)
    B = x.shape[0]
    consts = make_consts()
    per_core = B // N_CORES
    assert per_core == NB_CORE
    in_maps = [
        {"x": x[c * per_core : (c + 1) * per_core], **consts}
        for c in range(N_CORES)
    ]
    nc = _get_nc()
    res = run_bass_kernel_spmd(nc, in_maps, core_ids=list(range(N_CORES)))
    outs = []
    for r in res.results:
        o = r["out"].copy()  # [NB_CORE, 512, 4*NCH]
        o[:, :, 2 * NCH :] = o[:, ::-1, 2 * NCH :]
        outs.append(o)
    return np.concatenate(outs, axis=0)


# revision 30
# speedup vs baseline: 1.3378x; 1.3083x over previous
"""Trainium2 Bass kernel for DiagonalKernelAverageV2.

Math: for each (b, ch) image X [512, 512] and each of 4 corners, the output
at index i is the mean over the L-shaped shell of the i-th nested corner
square:  shell[i] = d[i] - d[i-1],  d[i] = sum of the (i+1)x(i+1) corner
window,  counts[i] = 2i+1.

Only two shell families are computed directly (top-left and top-right); the
bottom corners follow from row/col totals:
    shell_tl[i] = sum_{c<=i} X[i,c] + sum_{r<i}  X[r,i]
    shell_tr[i] = sum_{c>=511-i} X[i,c] + sum_{r<i} X[r,511-i]
    shell_br[i] = S[511-i] + ST[511-i] - shell_tl[511-i]
    shell_bl[i] = S[511-i] + ST[i]     - shell_tr[511-i]
(S = row sums, ST = col sums.)

Inputs are fed to the device as fp16 (quantization rel-err ~2e-4, well under
the 2e-2 gate); this halves HBM traffic and runs every PE matmul at the
1-cycle/row rate.

Per-core layout: batch-sharded (4 batches x 8 channels per core).  Each image
is 4 row-tiles [128, 512], held as 24 x [128, 128] blocks per image in XP:
blocks 0-15 = X (t-major), 16-19 = P1 (strict-upper-masked diagonal blocks),
20-23 = P2rev (strict-upper-masked reversed antidiagonal blocks).  Work split
by engine, per pair of images:
  - GpSimd: masked products (writes XP blocks 16-23); tail of the block-sum
    tree (widths 16 -> 1) producing B24 = 16 block sums + RSsu + RS2su per
    image; all per-batch assembly and output weighting.
  - VectorE: first two levels of the pairwise-add block-sum tree (fp16 2x).
  - TensorE: per-tile matmuls with constant-column weights accumulate column
    prefix sums / totals; ones-matmuls over P1/P2rev give the within-block
    column partial sums; transposes (incl. reversed-stride views) move
    column-indexed rows onto partitions.
  - ScalarE: PSUM->SBUF staging copies.
Bottom-corner outputs are written in source order and flipped on the host.
"""

import numpy as np

SIZE = 512
NT = 4  # row tiles per image
NCH = 8  # channels per batch
NB_CORE = 4  # batches per core
N_CORES = 8
NQ = 10  # transposed quantity cols per tile: 6 fwd + 4 reversed
NPAIR = NCH // 2


def build_nc():
    import concourse.bass as bass
    import concourse.bacc as bacc
    import concourse.mybir as mybir
    from concourse.tile import TileContext

    f32 = mybir.dt.float32
    f16 = mybir.dt.float16
    nc = bacc.Bacc()

    x = nc.dram_tensor("x", [NB_CORE, NCH, SIZE, SIZE], f16, kind="ExternalInput")
    msu_d = nc.dram_tensor("msu", [128, 8 * 128], f16, kind="ExternalInput")
    vw_d = nc.dram_tensor("vw", [128, 36], f16, kind="ExternalInput")
    eye_d = nc.dram_tensor("eye", [128, 128], f32, kind="ExternalInput")
    wg_d = nc.dram_tensor("wg", [128, NCH, NT], f32, kind="ExternalInput")
    wrevg_d = nc.dram_tensor("wrevg", [128, NCH, NT], f32, kind="ExternalInput")
    out = nc.dram_tensor("out", [NB_CORE, SIZE, 4 * NCH], f32, kind="ExternalOutput")

    ADD = mybir.AluOpType.add
    MULT = mybir.AluOpType.mult
    SUB = mybir.AluOpType.subtract

    with TileContext(nc) as tc, nc.allow_low_precision(reason="fp16 pipeline"):
        with (
            tc.tile_pool(name="consts", bufs=1) as consts,
            tc.tile_pool(name="xs", bufs=2) as xpool,
            tc.tile_pool(name="tree", bufs=2) as trpool,
            tc.tile_pool(name="tin", bufs=2) as tinpool,
            tc.tile_pool(name="perb", bufs=2) as bpool,
            tc.tile_pool(name="small", bufs=2) as spool,
            tc.tile_pool(name="psq", bufs=3, space="PSUM") as psq,
            tc.tile_pool(name="pst", bufs=2, space="PSUM") as pst,
        ):
            msu = consts.tile([128, 8 * 128], f16)
            nc.sync.dma_start(out=msu, in_=msu_d[:])
            vw = consts.tile([128, 36], f16)
            nc.sync.dma_start(out=vw, in_=vw_d[:])
            eye = consts.tile([128, 128], f32)
            nc.sync.dma_start(out=eye, in_=eye_d[:])
            wg = consts.tile([128, NCH, NT], f32)
            nc.sync.dma_start(out=wg, in_=wg_d[:])
            wrevg = consts.tile([128, NCH, NT], f32)
            nc.sync.dma_start(out=wrevg, in_=wrevg_d[:])
            msu8 = msu.rearrange("p (i t c) -> p i t c", i=2, c=128)

            from concourse.bass import _add_dep_helper

            prev_pe_last = None
            for b in range(NB_CORE):
                # B24[p, g, k]: k=4t+j -> block sum B[t][j]; k=16+t -> RSsu[t];
                # k=20+t -> RS2su[t]
                B24 = bpool.tile([128, NCH, 24], f32, tag="b24")
                TQ = bpool.tile([128, NCH, NT, NQ], f32, tag="tq")

                # one input DMA per batch: 8 images (APs merge to 3D)
                staged = []
                X8 = xpool.tile([128, NCH, NT, SIZE], f16, tag="x8")
                nc.sync.dma_start(
                    out=X8,
                    in_=x[b].rearrange("i (t p) c -> p i t c", p=128),
                )

                for gp in range(NPAIR):
                    Xpr = X8[:, 2 * gp : 2 * gp + 2]  # [128, 2, NT, SIZE]
                    x0 = Xpr[:, 0, 0, 0:1]

                    def blk_ap(base, tstep, cstep=1, coff=0):
                        # [p][i(2)][t(4)][c(128)] over the pair
                        return bass.AP(
                            tensor=X8.tensor,
                            offset=x0.offset + base * 128 + coff,
                            ap=[x0.ap[0]]
                            + [[NT * SIZE, 2], [tstep * 128, NT], [cstep, 128]],
                        )

                    # masked products on GpSimd -> PP (blocks 0-3 = P1,
                    # 4-7 = P2rev, per image)
                    PP = xpool.tile([128, 2, 8, 128], f16, tag="pp")
                    nc.gpsimd.tensor_tensor(
                        PP[:, :, 0:4, :], blk_ap(0, 5), msu8, op=MULT
                    )
                    nc.gpsimd.tensor_tensor(
                        PP[:, :, 4:8, :],
                        blk_ap(3, 3, cstep=-1, coff=127),
                        msu8,
                        op=MULT,
                    )

                    # block row sums: fp16 2x pairwise-add tree; first two
                    # levels on VectorE, tail on GpSimd.  T* blocks 0-15 = X,
                    # 16-23 = PP.
                    T1 = trpool.tile([128, 2, 24, 64], f16, tag="t1")
                    T2 = trpool.tile([128, 2, 24, 32], f16, tag="t2")
                    T3 = trpool.tile([128, 2, 24, 16], f16, tag="t3")
                    T4 = trpool.tile([128, 2, 24, 8], f16, tag="t4")
                    T5 = trpool.tile([128, 2, 24, 4], f16, tag="t5")
                    T6 = trpool.tile([128, 2, 24, 2], f16, tag="t6")
                    Xblk = Xpr.rearrange("p i t (j c) -> p i (t j) c", c=128)
                    nc.vector.tensor_tensor(
                        T1[:, :, 0:16, :], Xblk[:, :, :, 0:64],
                        Xblk[:, :, :, 64:128], op=ADD,
                    )
                    nc.vector.tensor_tensor(
                        T1[:, :, 16:24, :], PP[:, :, :, 0:64],
                        PP[:, :, :, 64:128], op=ADD,
                    )
                    nc.vector.tensor_tensor(
                        T2, T1[:, :, :, 0:32], T1[:, :, :, 32:64], op=ADD
                    )
                    nc.gpsimd.tensor_tensor(
                        T3, T2[:, :, :, 0:16], T2[:, :, :, 16:32], op=ADD
                    )
                    nc.gpsimd.tensor_tensor(
                        T4, T3[:, :, :, 0:8], T3[:, :, :, 8:16], op=ADD
                    )
                    nc.gpsimd.tensor_tensor(
                        T5, T4[:, :, :, 0:4], T4[:, :, :, 4:8], op=ADD
                    )
                    nc.gpsimd.tensor_tensor(
                        T6, T5[:, :, :, 0:2], T5[:, :, :, 2:4], op=ADD
                    )
                    nc.gpsimd.tensor_tensor(
                        B24[:, 2 * gp : 2 * gp + 2],
                        T6[:, :, :, 0],
                        T6[:, :, :, 1],
                        op=ADD,
                    )

                    # column-side quantities on PE: per image one accum group;
                    # rows 0-2: CPfx[1..3], 3: ST, 4: colsum(P1), 5: colsum(P2rev).
                    # Both matmul groups of the pair run before either image's
                    # transposes so PE is not stalled on the PSUM->SBUF staging.
                    psumQs = []
                    for i in (0, 1):
                        XPi = Xpr[:, i].rearrange("p a b -> p (a b)")
                        PPi = PP[:, i].rearrange("p a b -> p (a b)")
                        psumQ = psq.tile([6, SIZE], f32)
                        for t in range(NT):
                            mm = nc.tensor.matmul(
                                psumQ[0:6, :],
                                lhsT=vw[:, 6 * t : 6 * t + 6],
                                rhs=XPi[:, 512 * t : 512 * (t + 1)],
                                start=(t == 0),
                                stop=False,
                            )
                            # keep PE program order: no transpose-mode matmul
                            # and no other accum group may interleave here
                            if t == 0 and prev_pe_last is not None:
                                _add_dep_helper(
                                    mm.ins, prev_pe_last.ins, sync=False,
                                    reason="PE group ordering",
                                )
                        nc.tensor.matmul(
                            psumQ[0:6, :], lhsT=vw[:, 24:30],
                            rhs=PPi[:, 0:512], start=False, stop=False,
                        )
                        prev_pe_last = nc.tensor.matmul(
                            psumQ[0:6, :], lhsT=vw[:, 30:36],
                            rhs=PPi[:, 512:1024], start=False, stop=True,
                        )
                        psumQs.append(psumQ)
                        # staging on ScalarE overlaps later matmul groups
                        Tin = tinpool.tile([6, SIZE], f32, tag=f"tin{gp}{i}")
                        TinB = tinpool.tile([4, SIZE], f32, tag=f"tinb{gp}{i}")
                        nc.scalar.copy(Tin[0:6, :], psumQ[0:6, :])
                        nc.scalar.copy(TinB[0:4, :], psumQ[0:4, ::-1])
                        staged.append((2 * gp + i, Tin, TinB))

                # all transposes after the batch's matmul groups: staging is
                # long done, so PE never stalls on the PSUM->SBUF copies
                for g, Tin, TinB in staged:
                    psumT = pst.tile([128, NT * NQ], f32)
                    for t in range(NT):
                        tr = nc.tensor.transpose(
                            psumT[:, NQ * t : NQ * t + 6],
                            in_=Tin[0:6, 128 * t : 128 * (t + 1)],
                            identity=eye[0:6, 0:6],
                        )
                        if t == 0:
                            _add_dep_helper(
                                tr.ins, prev_pe_last.ins, sync=False,
                                reason="PE group ordering",
                            )
                        prev_pe_last = nc.tensor.transpose(
                            psumT[:, NQ * t + 6 : NQ * t + 10],
                            in_=TinB[0:4, 128 * t : 128 * (t + 1)],
                            identity=eye[0:4, 0:4],
                        )
                    nc.scalar.copy(
                        TQ[:, g].rearrange("p t q -> p (t q)"), psumT[:, :]
                    )

                # ---- per-batch assembly on GpSimd ([128, (g), (t)] ops) ----
                def bg_ap(base, tstep):
                    return bass.AP(
                        tensor=B24.tensor,
                        offset=B24[:, 0, 0:1].offset + base,
                        ap=[B24[:, 0, 0:1].ap[0]] + [[24, NCH], [tstep, NT]],
                    )

                def tq_ap(base, tstep, nt=NT):
                    return bass.AP(
                        tensor=TQ.tensor,
                        offset=TQ[:, 0, 0, 0:1].offset + base,
                        ap=[TQ[:, 0, 0, 0:1].ap[0]] + [[NT * NQ, NCH], [tstep, nt]],
                    )

                PI = bpool.tile([128, NCH, 5, NT], f32, tag="pi")

                def pi_ap(base, tstep, nt=NT):
                    return bass.AP(
                        tensor=PI.tensor,
                        offset=PI[:, 0, 0, 0:1].offset + base,
                        ap=[PI[:, 0, 0, 0:1].ap[0]] + [[20, NCH], [tstep, nt]],
                    )

                nc.gpsimd.memset(PI[:, :, 0, :], 0.0)
                nc.gpsimd.tensor_copy(PI[:, :, 1, :], bg_ap(0, 4))
                for m in range(2, 5):
                    nc.gpsimd.tensor_tensor(
                        PI[:, :, m, :], PI[:, :, m - 1, :], bg_ap(m - 1, 4),
                        op=ADD,
                    )

                sh_tl = spool.tile([128, NCH, NT], f32, tag="shtl")
                sh_tr = spool.tile([128, NCH, NT], f32, tag="shtr")
                # shell_tl = B[t][t] - RSsu + PI[m=t] + CPfx[m=t] + CS1
                nc.gpsimd.tensor_tensor(sh_tl, bg_ap(0, 5), bg_ap(16, 1), op=SUB)
                nc.gpsimd.tensor_tensor(sh_tl, sh_tl, pi_ap(0, 5), op=ADD)
                nc.gpsimd.tensor_tensor(
                    sh_tl[:, :, 1:4], sh_tl[:, :, 1:4], tq_ap(NQ, NQ + 1, 3), op=ADD
                )
                nc.gpsimd.tensor_tensor(sh_tl, sh_tl, tq_ap(4, NQ), op=ADD)
                # shell_tr = B[t][3-t] - RS2su + S - PI[m=4-t] + CPfxRev[m=t] + CS2
                nc.gpsimd.tensor_tensor(sh_tr, bg_ap(3, 3), bg_ap(20, 1), op=SUB)
                nc.gpsimd.tensor_tensor(sh_tr, sh_tr, pi_ap(16, 1), op=ADD)
                nc.gpsimd.tensor_tensor(sh_tr, sh_tr, pi_ap(16, -3), op=SUB)
                nc.gpsimd.tensor_tensor(
                    sh_tr[:, :, 1:4], sh_tr[:, :, 1:4], tq_ap(NQ + 6, NQ + 1, 3),
                    op=ADD,
                )
                nc.gpsimd.tensor_tensor(sh_tr, sh_tr, tq_ap(5, NQ), op=ADD)

                # br (src order): u = ST - shell_tl + S ; bl: v = STrev - shell_tr + S
                u = spool.tile([128, NCH, NT], f32, tag="u")
                v = spool.tile([128, NCH, NT], f32, tag="v")
                nc.gpsimd.tensor_tensor(u, tq_ap(3, NQ), sh_tl, op=SUB)
                nc.gpsimd.tensor_tensor(u, u, pi_ap(16, 1), op=ADD)
                nc.gpsimd.tensor_tensor(v, tq_ap(9, NQ), sh_tr, op=SUB)
                nc.gpsimd.tensor_tensor(v, v, pi_ap(16, 1), op=ADD)
                # outputs as one [128, t, ci, g] tile, weighted; one DMA per batch
                o_all = spool.tile([128, NT, 4, NCH], f32, tag="oall")
                for ci, (src, wt) in enumerate(
                    [(sh_tl, wg), (sh_tr, wg), (v, wrevg), (u, wrevg)]
                ):
                    nc.gpsimd.tensor_tensor(
                        o_all[:, :, ci, :],
                        src.rearrange("p g t -> p t g"),
                        wt.rearrange("p g t -> p t g"),
                        op=MULT,
                    )
                nc.sync.dma_start(
                    out=out[b].rearrange("(t p) c -> p t c", p=128),
                    in_=o_all.rearrange("p t c g -> p t (c g)"),
                )
    nc.compile()
    return nc


def make_consts():
    r = np.arange(128)
    msu = np.tile((r[None, :] > r[:, None]).astype(np.float16), (1, 8))  # [c > r]
    vw = np.zeros((128, 36), np.float16)
    for t in range(NT):
        for m in range(3):
            vw[:, 6 * t + m] = 1.0 if t < m + 1 else 0.0  # CPfx[m+1]
        vw[:, 6 * t + 3] = 1.0  # ST
    vw[:, 24 + 4] = 1.0  # colsum(P1) -> row 4
    vw[:, 30 + 5] = 1.0  # colsum(P2rev) -> row 5
    eye = np.eye(128, dtype=np.float32)
    i_pt = (r[:, None] + 128 * np.arange(NT)[None, :]).astype(np.float64)
    w_pt = (1.0 / (2 * i_pt + 1)).astype(np.float32)  # [128, NT]
    wrev_pt = (1.0 / (1023.0 - 2 * i_pt)).astype(np.float32)
    wg = np.tile(w_pt[:, None, :], (1, NCH, 1)).astype(np.float32)
    wrevg = np.tile(wrev_pt[:, None, :], (1, NCH, 1)).astype(np.float32)
    return dict(msu=msu, vw=vw, eye=eye, wg=wg, wrevg=wrevg)


_NC = None


def _get_nc():
    global _NC
    if _NC is None:
        _NC = build_nc()
    return _NC


def kernel(x: np.ndarray) -> np.ndarray:
    from concourse.bass_utils import run_bass_kernel_spmd

    x = np.asarray(x, dtype=np.float32).astype(np.float16)
    B = x.shape[0]
    consts = make_consts()
    per_core = B // N_CORES
    assert per_core == NB_CORE
    in_maps = [
        {"x": x[c * per_core : (c + 1) * per_core], **consts}
        for c in range(N_CORES)
    ]
    nc = _get_nc()
    res = run_bass_kernel_spmd(nc, in_maps, core_ids=list(range(N_CORES)))
    outs = []
    for r in res.results:
        o = r["out"].copy()  # [NB_CORE, 512, 4*NCH]
        o[:, :, 2 * NCH :] = o[:, ::-1, 2 * NCH :]
        outs.append(o)
    return np.concatenate(outs, axis=0)


# revision 31
# speedup vs baseline: 1.5266x; 1.1411x over previous
"""Trainium2 Bass kernel for DiagonalKernelAverageV2.

Math: for each (b, ch) image X [512, 512] and each of 4 corners, the output
at index i is the mean over the L-shaped shell of the i-th nested corner
square:  shell[i] = d[i] - d[i-1],  d[i] = sum of the (i+1)x(i+1) corner
window,  counts[i] = 2i+1.

Only two shell families are computed directly (top-left and top-right); the
bottom corners follow from row/col totals:
    shell_tl[i] = sum_{c<=i} X[i,c] + sum_{r<i}  X[r,i]
    shell_tr[i] = sum_{c>=511-i} X[i,c] + sum_{r<i} X[r,511-i]
    shell_br[i] = S[511-i] + ST[511-i] - shell_tl[511-i]
    shell_bl[i] = S[511-i] + ST[i]     - shell_tr[511-i]
(S = row sums, ST = col sums.)

Inputs are fed to the device as fp16 (quantization rel-err ~2e-4, well under
the 2e-2 gate); this halves HBM traffic and runs every PE matmul at the
1-cycle/row rate.

Per-core layout: batch-sharded (4 batches x 8 channels per core).  Each image
is 4 row-tiles [128, 512], held as 24 x [128, 128] blocks per image in XP:
blocks 0-15 = X (t-major), 16-19 = P1 (strict-upper-masked diagonal blocks),
20-23 = P2rev (strict-upper-masked reversed antidiagonal blocks).  Work split
by engine, per pair of images:
  - GpSimd: masked products (writes XP blocks 16-23); tail of the block-sum
    tree (widths 16 -> 1) producing B24 = 16 block sums + RSsu + RS2su per
    image; all per-batch assembly and output weighting.
  - VectorE: first two levels of the pairwise-add block-sum tree (fp16 2x).
  - TensorE: per-tile matmuls with constant-column weights accumulate column
    prefix sums / totals; ones-matmuls over P1/P2rev give the within-block
    column partial sums; transposes (incl. reversed-stride views) move
    column-indexed rows onto partitions.
  - ScalarE: PSUM->SBUF staging copies.
Bottom-corner outputs are written in source order and flipped on the host.
"""

import numpy as np

SIZE = 512
NT = 4  # row tiles per image
NCH = 8  # channels per batch
NB_CORE = 4  # batches per core
N_CORES = 8
NQ = 10  # transposed quantity cols per tile: 6 fwd + 4 reversed
NPAIR = NCH // 2


def build_nc():
    import concourse.bass as bass
    import concourse.bacc as bacc
    import concourse.mybir as mybir
    from concourse.tile import TileContext

    f32 = mybir.dt.float32
    f16 = mybir.dt.float16
    nc = bacc.Bacc()

    x = nc.dram_tensor("x", [NB_CORE, NCH, SIZE, SIZE], f16, kind="ExternalInput")
    msu_d = nc.dram_tensor("msu", [128, 8 * 128], f16, kind="ExternalInput")
    vw_d = nc.dram_tensor("vw", [128, 36], f16, kind="ExternalInput")
    eye_d = nc.dram_tensor("eye", [128, 128], f32, kind="ExternalInput")
    wg_d = nc.dram_tensor("wg", [128, NCH, NT], f32, kind="ExternalInput")
    wrevg_d = nc.dram_tensor("wrevg", [128, NCH, NT], f32, kind="ExternalInput")
    out = nc.dram_tensor("out", [NB_CORE, SIZE, 4 * NCH], f32, kind="ExternalOutput")

    ADD = mybir.AluOpType.add
    MULT = mybir.AluOpType.mult
    SUB = mybir.AluOpType.subtract

    with TileContext(nc) as tc, nc.allow_low_precision(reason="fp16 pipeline"):
        with (
            tc.tile_pool(name="consts", bufs=1) as consts,
            tc.tile_pool(name="xs", bufs=2) as xpool,
            tc.tile_pool(name="tree", bufs=2) as trpool,
            tc.tile_pool(name="tin", bufs=2) as tinpool,
            tc.tile_pool(name="perb", bufs=2) as bpool,
            tc.tile_pool(name="small", bufs=2) as spool,
            tc.tile_pool(name="psq", bufs=3, space="PSUM") as psq,
            tc.tile_pool(name="pst", bufs=2, space="PSUM") as pst,
        ):
            msu = consts.tile([128, 8 * 128], f16)
            nc.sync.dma_start(out=msu, in_=msu_d[:])
            vw = consts.tile([128, 36], f16)
            nc.sync.dma_start(out=vw, in_=vw_d[:])
            eye = consts.tile([128, 128], f32)
            nc.sync.dma_start(out=eye, in_=eye_d[:])
            wg = consts.tile([128, NCH, NT], f32)
            nc.sync.dma_start(out=wg, in_=wg_d[:])
            wrevg = consts.tile([128, NCH, NT], f32)
            nc.sync.dma_start(out=wrevg, in_=wrevg_d[:])
            msu8 = msu.rearrange("p (i t c) -> p i t c", i=2, c=128)

            from concourse.bass import _add_dep_helper

            prev_pe_last = None
            for b in range(NB_CORE):
                # B24[p, g, k]: k=4t+j -> block sum B[t][j]; k=16+t -> RSsu[t];
                # k=20+t -> RS2su[t]
                B24 = bpool.tile([128, NCH, 24], f32, tag="b24")
                TQ = bpool.tile([128, NCH, NT, NQ], f32, tag="tq")

                # input DMAs at pair granularity (finer pipelining, less
                # head-of-line latency; APs merge to 3D)
                staged = []
                X8 = xpool.tile([128, NCH, NT, SIZE], f16, tag="x8")
                for gp in range(NPAIR):
                    nc.sync.dma_start(
                        out=X8[:, 2 * gp : 2 * gp + 2],
                        in_=x[b, 2 * gp : 2 * gp + 2].rearrange(
                            "i (t p) c -> p i t c", p=128
                        ),
                    )

                for gp in range(NPAIR):
                    Xpr = X8[:, 2 * gp : 2 * gp + 2]  # [128, 2, NT, SIZE]
                    x0 = Xpr[:, 0, 0, 0:1]

                    def blk_ap(base, tstep, cstep=1, coff=0):
                        # [p][i(2)][t(4)][c(128)] over the pair
                        return bass.AP(
                            tensor=X8.tensor,
                            offset=x0.offset + base * 128 + coff,
                            ap=[x0.ap[0]]
                            + [[NT * SIZE, 2], [tstep * 128, NT], [cstep, 128]],
                        )

                    # masked products on GpSimd -> PP (blocks 0-3 = P1,
                    # 4-7 = P2rev, per image)
                    PP = xpool.tile([128, 2, 8, 128], f16, tag="pp")
                    nc.gpsimd.tensor_tensor(
                        PP[:, :, 0:4, :], blk_ap(0, 5), msu8, op=MULT
                    )
                    nc.gpsimd.tensor_tensor(
                        PP[:, :, 4:8, :],
                        blk_ap(3, 3, cstep=-1, coff=127),
                        msu8,
                        op=MULT,
                    )

                    # block row sums: fp16 2x pairwise-add tree; first two
                    # levels on VectorE, tail on GpSimd.  T* blocks 0-15 = X,
                    # 16-23 = PP.
                    T1 = trpool.tile([128, 2, 24, 64], f16, tag="t1")
                    T2 = trpool.tile([128, 2, 24, 32], f16, tag="t2")
                    T3 = trpool.tile([128, 2, 24, 16], f16, tag="t3")
                    T4 = trpool.tile([128, 2, 24, 8], f16, tag="t4")
                    T5 = trpool.tile([128, 2, 24, 4], f16, tag="t5")
                    T6 = trpool.tile([128, 2, 24, 2], f16, tag="t6")
                    Xblk = Xpr.rearrange("p i t (j c) -> p i (t j) c", c=128)
                    nc.vector.tensor_tensor(
                        T1[:, :, 0:16, :], Xblk[:, :, :, 0:64],
                        Xblk[:, :, :, 64:128], op=ADD,
                    )
                    nc.vector.tensor_tensor(
                        T1[:, :, 16:24, :], PP[:, :, :, 0:64],
                        PP[:, :, :, 64:128], op=ADD,
                    )
                    nc.vector.tensor_tensor(
                        T2, T1[:, :, :, 0:32], T1[:, :, :, 32:64], op=ADD
                    )
                    nc.gpsimd.tensor_tensor(
                        T3, T2[:, :, :, 0:16], T2[:, :, :, 16:32], op=ADD
                    )
                    nc.gpsimd.tensor_tensor(
                        T4, T3[:, :, :, 0:8], T3[:, :, :, 8:16], op=ADD
                    )
                    nc.gpsimd.tensor_tensor(
                        T5, T4[:, :, :, 0:4], T4[:, :, :, 4:8], op=ADD
                    )
                    nc.gpsimd.tensor_tensor(
                        T6, T5[:, :, :, 0:2], T5[:, :, :, 2:4], op=ADD
                    )
                    nc.gpsimd.tensor_tensor(
                        B24[:, 2 * gp : 2 * gp + 2],
                        T6[:, :, :, 0],
                        T6[:, :, :, 1],
                        op=ADD,
                    )

                    # column-side quantities on PE: per image one accum group;
                    # rows 0-2: CPfx[1..3], 3: ST, 4: colsum(P1), 5: colsum(P2rev).
                    # Both matmul groups of the pair run before either image's
                    # transposes so PE is not stalled on the PSUM->SBUF staging.
                    psumQs = []
                    for i in (0, 1):
                        XPi = Xpr[:, i].rearrange("p a b -> p (a b)")
                        PPi = PP[:, i].rearrange("p a b -> p (a b)")
                        psumQ = psq.tile([6, SIZE], f32)
                        for t in range(NT):
                            mm = nc.tensor.matmul(
                                psumQ[0:6, :],
                                lhsT=vw[:, 6 * t : 6 * t + 6],
                                rhs=XPi[:, 512 * t : 512 * (t + 1)],
                                start=(t == 0),
                                stop=False,
                            )
                            # keep PE program order: no transpose-mode matmul
                            # and no other accum group may interleave here
                            if t == 0 and prev_pe_last is not None:
                                _add_dep_helper(
                                    mm.ins, prev_pe_last.ins, sync=False,
                                    reason="PE group ordering",
                                )
                        nc.tensor.matmul(
                            psumQ[0:6, :], lhsT=vw[:, 24:30],
                            rhs=PPi[:, 0:512], start=False, stop=False,
                        )
                        prev_pe_last = nc.tensor.matmul(
                            psumQ[0:6, :], lhsT=vw[:, 30:36],
                            rhs=PPi[:, 512:1024], start=False, stop=True,
                        )
                        psumQs.append(psumQ)
                        # staging on ScalarE overlaps later matmul groups
                        Tin = tinpool.tile([6, SIZE], f32, tag=f"tin{gp}{i}")
                        TinB = tinpool.tile([4, SIZE], f32, tag=f"tinb{gp}{i}")
                        nc.scalar.copy(Tin[0:6, :], psumQ[0:6, :])
                        nc.scalar.copy(TinB[0:4, :], psumQ[0:4, ::-1])
                        staged.append((2 * gp + i, Tin, TinB))

                # all transposes after the batch's matmul groups: staging is
                # long done, so PE never stalls on the PSUM->SBUF copies
                for g, Tin, TinB in staged:
                    psumT = pst.tile([128, NT * NQ], f32)
                    for t in range(NT):
                        tr = nc.tensor.transpose(
                            psumT[:, NQ * t : NQ * t + 6],
                            in_=Tin[0:6, 128 * t : 128 * (t + 1)],
                            identity=eye[0:6, 0:6],
                        )
                        if t == 0:
                            _add_dep_helper(
                                tr.ins, prev_pe_last.ins, sync=False,
                                reason="PE group ordering",
                            )
                        prev_pe_last = nc.tensor.transpose(
                            psumT[:, NQ * t + 6 : NQ * t + 10],
                            in_=TinB[0:4, 128 * t : 128 * (t + 1)],
                            identity=eye[0:4, 0:4],
                        )
                    nc.scalar.copy(
                        TQ[:, g].rearrange("p t q -> p (t q)"), psumT[:, :]
                    )

                # ---- per-batch assembly on GpSimd ([128, (g), (t)] ops) ----
                def bg_ap(base, tstep):
                    return bass.AP(
                        tensor=B24.tensor,
                        offset=B24[:, 0, 0:1].offset + base,
                        ap=[B24[:, 0, 0:1].ap[0]] + [[24, NCH], [tstep, NT]],
                    )

                def tq_ap(base, tstep, nt=NT):
                    return bass.AP(
                        tensor=TQ.tensor,
                        offset=TQ[:, 0, 0, 0:1].offset + base,
                        ap=[TQ[:, 0, 0, 0:1].ap[0]] + [[NT * NQ, NCH], [tstep, nt]],
                    )

                PI = bpool.tile([128, NCH, 5, NT], f32, tag="pi")

                def pi_ap(base, tstep, nt=NT):
                    return bass.AP(
                        tensor=PI.tensor,
                        offset=PI[:, 0, 0, 0:1].offset + base,
                        ap=[PI[:, 0, 0, 0:1].ap[0]] + [[20, NCH], [tstep, nt]],
                    )

                nc.gpsimd.memset(PI[:, :, 0, :], 0.0)
                nc.gpsimd.tensor_copy(PI[:, :, 1, :], bg_ap(0, 4))
                for m in range(2, 5):
                    nc.gpsimd.tensor_tensor(
                        PI[:, :, m, :], PI[:, :, m - 1, :], bg_ap(m - 1, 4),
                        op=ADD,
                    )

                sh_tl = spool.tile([128, NCH, NT], f32, tag="shtl")
                sh_tr = spool.tile([128, NCH, NT], f32, tag="shtr")
                # shell_tl = B[t][t] - RSsu + PI[m=t] + CPfx[m=t] + CS1
                nc.gpsimd.tensor_tensor(sh_tl, bg_ap(0, 5), bg_ap(16, 1), op=SUB)
                nc.gpsimd.tensor_tensor(sh_tl, sh_tl, pi_ap(0, 5), op=ADD)
                nc.gpsimd.tensor_tensor(
                    sh_tl[:, :, 1:4], sh_tl[:, :, 1:4], tq_ap(NQ, NQ + 1, 3), op=ADD
                )
                nc.gpsimd.tensor_tensor(sh_tl, sh_tl, tq_ap(4, NQ), op=ADD)
                # shell_tr = B[t][3-t] - RS2su + S - PI[m=4-t] + CPfxRev[m=t] + CS2
                nc.gpsimd.tensor_tensor(sh_tr, bg_ap(3, 3), bg_ap(20, 1), op=SUB)
                nc.gpsimd.tensor_tensor(sh_tr, sh_tr, pi_ap(16, 1), op=ADD)
                nc.gpsimd.tensor_tensor(sh_tr, sh_tr, pi_ap(16, -3), op=SUB)
                nc.gpsimd.tensor_tensor(
                    sh_tr[:, :, 1:4], sh_tr[:, :, 1:4], tq_ap(NQ + 6, NQ + 1, 3),
                    op=ADD,
                )
                nc.gpsimd.tensor_tensor(sh_tr, sh_tr, tq_ap(5, NQ), op=ADD)

                # br (src order): u = ST - shell_tl + S ; bl: v = STrev - shell_tr + S
                u = spool.tile([128, NCH, NT], f32, tag="u")
                v = spool.tile([128, NCH, NT], f32, tag="v")
                nc.gpsimd.tensor_tensor(u, tq_ap(3, NQ), sh_tl, op=SUB)
                nc.gpsimd.tensor_tensor(u, u, pi_ap(16, 1), op=ADD)
                nc.gpsimd.tensor_tensor(v, tq_ap(9, NQ), sh_tr, op=SUB)
                nc.gpsimd.tensor_tensor(v, v, pi_ap(16, 1), op=ADD)
                # outputs as one [128, t, ci, g] tile, weighted; one DMA per batch
                o_all = spool.tile([128, NT, 4, NCH], f32, tag="oall")
                for ci, (src, wt) in enumerate(
                    [(sh_tl, wg), (sh_tr, wg), (v, wrevg), (u, wrevg)]
                ):
                    nc.gpsimd.tensor_tensor(
                        o_all[:, :, ci, :],
                        src.rearrange("p g t -> p t g"),
                        wt.rearrange("p g t -> p t g"),
                        op=MULT,
                    )
                nc.sync.dma_start(
                    out=out[b].rearrange("(t p) c -> p t c", p=128),
                    in_=o_all.rearrange("p t c g -> p t (c g)"),
                )
    nc.compile()
    return nc


def make_consts():
    r = np.arange(128)
    msu = np.tile((r[None, :] > r[:, None]).astype(np.float16), (1, 8))  # [c > r]
    vw = np.zeros((128, 36), np.float16)
    for t in range(NT):
        for m in range(3):
            vw[:, 6 * t + m] = 1.0 if t < m + 1 else 0.0  # CPfx[m+1]
        vw[:, 6 * t + 3] = 1.0  # ST
    vw[:, 24 + 4] = 1.0  # colsum(P1) -> row 4
    vw[:, 30 + 5] = 1.0  # colsum(P2rev) -> row 5
    eye = np.eye(128, dtype=np.float32)
    i_pt = (r[:, None] + 128 * np.arange(NT)[None, :]).astype(np.float64)
    w_pt = (1.0 / (2 * i_pt + 1)).astype(np.float32)  # [128, NT]
    wrev_pt = (1.0 / (1023.0 - 2 * i_pt)).astype(np.float32)
    wg = np.tile(w_pt[:, None, :], (1, NCH, 1)).astype(np.float32)
    wrevg = np.tile(wrev_pt[:, None, :], (1, NCH, 1)).astype(np.float32)
    return dict(msu=msu, vw=vw, eye=eye, wg=wg, wrevg=wrevg)


_NC = None


def _get_nc():
    global _NC
    if _NC is None:
        _NC = build_nc()
    return _NC


def kernel(x: np.ndarray) -> np.ndarray:
    from concourse.bass_utils import run_bass_kernel_spmd

    x = np.asarray(x, dtype=np.float32).astype(np.float16)
    B = x.shape[0]
    consts = make_consts()
    per_core = B // N_CORES
    assert per_core == NB_CORE
    in_maps = [
        {"x": x[c * per_core : (c + 1) * per_core], **consts}
        for c in range(N_CORES)
    ]
    nc = _get_nc()
    res = run_bass_kernel_spmd(nc, in_maps, core_ids=list(range(N_CORES)))
    outs = []
    for r in res.results:
        o = r["out"].copy()  # [NB_CORE, 512, 4*NCH]
        o[:, :, 2 * NCH :] = o[:, ::-1, 2 * NCH :]
        outs.append(o)
    return np.concatenate(outs, axis=0)
